# revision 15
# baseline (speedup 1.0000x reference)
"""MixIT loss kernel for Trainium2 (8 NeuronCores, Bass/Tile) — v6.

Math: reference computes, for each of 16 assignment combinations k,
    mix[k,b,c,t] = sum_s A[k,c,s] * x[b,s,t]        (A tiny [16,2,4])
    loss[k] = sum_b [ snr(mix[k,b,0], m1[b]) + snr(mix[k,b,1], m2[b]) ]
and returns (argmin_k, min_k).  Everything reduces to the 6x6 Gram matrix
of the per-batch streams {x_0..x_3, m1, m2} over T=64000; the device
computes pairwise dot products, the host finishes the 16-way argmin.

v6 layout per core (4 batches = 24 streams; T = 128 partitions x 500
cols):
 - DMA (f32): m1/m2 as full-row transfers (2000B descriptors, ~94% HBM
   eff) on the scalar ring; x in tapered T-chunks on the sync ring so
   downstream stages can chase; the tiny last chunk shrinks the tail.
 - Re-layout: f32 tensor_copy za[p, j, c] -> zbt[p, c, j] (the fastest
   measured DVE pattern: strided 4B reads, contiguous 96B dst runs, 24
   lanes per op).  Work is split along c across DVE / GpSimd / ACT in
   per-chunk slices sized to each engine's measured rate; ACT gets one
   big op per chunk since each ACT op pays a ~1.4us pipe drain.
 - PE reads zbt as bf16 with NO cast: the operand is the high u16 half
   of each f32 word (= bf16 truncation, error < 0.4%, cancels in the
   final log-ratio; validated ~3e-8 end-to-end) via a uniform stride-2
   view — [128, 120] per group, so matmuls run 1-pass bf16 instead of
   2-pass fp32.  100 accumulating matmuls into 2 PSUM banks; bank A
   drains while the PE runs the small tail chunk.
 - Host sums the e-diagonal: G[j,k] = sum_banks sum_e out[(e,j),(e,k)].
"""

import itertools
import sys

import numpy as np

if "/opt/trn_rl_repo" not in sys.path:
    sys.path.insert(0, "/opt/trn_rl_repo")

N_CORES = 8
B = 32               # full batch
S = 4                # estimated sources
T = 64000
BL = B // N_CORES    # batches per core = 4
NJ = 6 * BL          # streams per core = 24 (16 x, 4 m1, 4 m2)
P = 128
COLS = T // P        # 500
FG = 5               # T-cols fused per matmul: FG*NJ = 120 <= 128
NG = COLS // FG      # 100 matmul groups
# T-chunks (cols, each % FG == 0, sum == COLS).  Tapered: big chunks
# amortize DMA descriptors, the small last chunk shrinks the PE tail.
X_CHUNKS = (140, 170, 105, 65, 20)
assert sum(X_CHUNKS) == COLS and all(c % FG == 0 for c in X_CHUNKS)
# Per-chunk copy split across engines, fractions of the chunk's columns;
# sized to measured rates DVE 2.0 / GpSimd 3.5 / ACT 3.6 ns/elem (ACT
# also issues 3 of the 4 chunk DMAs, so it gets a smaller share).
CP_DVE, CP_GPS = 0.50, 0.30           # ACT gets the rest
SNR_MAX = 30.0

_CACHE = {}
LAST_RESULTS = None  # BassKernelResults of the most recent run (for test harness)


def _build_nc():
    from concourse import bacc, bass, tile
    import concourse.mybir as mybir

    nc = bacc.Bacc("TRN2", target_bir_lowering=False, debug=False,
                   num_devices=N_CORES)
    f32 = mybir.dt.float32
    bf16 = mybir.dt.bfloat16
    x = nc.dram_tensor("x", [BL, S, T], f32, kind="ExternalInput")
    m1 = nc.dram_tensor("m1", [BL, T], f32, kind="ExternalInput")
    m2 = nc.dram_tensor("m2", [BL, T], f32, kind="ExternalInput")
    g = nc.dram_tensor("g", [2, FG * NJ, FG * NJ], f32, kind="ExternalOutput")

    grp_a_end = sum(X_CHUNKS[:-1]) // FG       # bank A: all but last chunk

    with tile.TileContext(nc) as tc:
        with (
            tc.tile_pool(name="za", bufs=1) as zapool,
            tc.tile_pool(name="zb", bufs=1) as zbpool,
            tc.tile_pool(name="ps", bufs=1, space=bass.MemorySpace.PSUM) as psp,
            tc.tile_pool(name="o", bufs=1) as opool,
        ):
            za = zapool.tile([P, NJ, COLS], f32, tag="za")
            zbt = zbpool.tile([P, COLS, NJ, 2], bf16, tag="zbt")
            zbf = zbt.bitcast(f32)             # [P, COLS, NJ, 1] f32 view
            acc_a = psp.tile([FG * NJ, FG * NJ], f32, tag="pa")
            acc_b = psp.tile([FG * NJ, FG * NJ], f32, tag="pb")

            xr = x.ap().rearrange("b s (p c) -> p (b s) c", p=P)
            m1r = m1.ap().rearrange("b (p c) -> p b c", p=P)
            m2r = m2.ap().rearrange("b (p c) -> p b c", p=P)
            # All tensors T-chunked; 12 streams per ring per chunk so both
            # HWDGE rings carry equal bytes with equal descriptor sizes and
            # a chunk's lanes complete together (packet round-robin shares
            # bandwidth by descriptor size).
            c0 = 0
            for cq in X_CHUNKS:
                sl = slice(c0, c0 + cq)
                nc.sync.dma_start(out=za[:, 0:12, sl], in_=xr[:, 0:12, sl])
                nc.scalar.dma_start(out=za[:, 12:16, sl],
                                    in_=xr[:, 12:16, sl])
                nc.scalar.dma_start(out=za[:, 16:20, sl], in_=m1r[:, :, sl])
                nc.scalar.dma_start(out=za[:, 20:24, sl], in_=m2r[:, :, sl])
                c0 += cq

            def cp(eng, c0, c1):
                if c1 <= c0:
                    return
                dst = zbf[:, c0:c1, :, 0]
                src = za[:, :, c0:c1].transpose([0, 2, 1])
                if eng is nc.scalar:
                    eng.copy(dst, src)
                else:
                    eng.tensor_copy(dst, src)

            # Copies chase each chunk; c-slices per engine, 24-lane ops.
            c0 = 0
            for cq in X_CHUNKS:
                dv = c0 + int(cq * CP_DVE)
                gp = dv + int(cq * CP_GPS)
                cp(nc.vector, c0, dv)
                cp(nc.gpsimd, dv, gp)
                cp(nc.scalar, gp, c0 + cq)
                c0 += cq

            for grp in range(NG):
                # High u16 halves of 120 consecutive f32: [128, 120] bf16
                # at uniform element stride 2 — a 1-pass bf16 matmul.
                op = zbt[:, FG * grp:FG * (grp + 1), :, 1]
                acc = acc_a if grp < grp_a_end else acc_b
                nc.tensor.matmul(
                    acc[:, :], op, op,
                    start=(grp == 0 or grp == grp_a_end),
                    stop=(grp == grp_a_end - 1 or grp == NG - 1),
                )
                if grp == grp_a_end - 1:
                    # bank A done: drain it while the PE runs the tail chunk
                    gout_a = opool.tile([FG * NJ, FG * NJ], f32, tag="oa")
                    nc.vector.tensor_copy(gout_a[:, :], acc_a[:, :])
                    nc.sync.dma_start(out=g.ap()[0], in_=gout_a[:, :])
            gout_b = opool.tile([FG * NJ, FG * NJ], f32, tag="ob")
            nc.vector.tensor_copy(gout_b[:, :], acc_b[:, :])
            nc.sync.dma_start(out=g.ap()[1], in_=gout_b[:, :])
    nc.compile()
    return nc


def _get_nc():
    if "nc" not in _CACHE:
        _CACHE["nc"] = _build_nc()
    return _CACHE["nc"]


def _finish_host(grams: np.ndarray):
    """grams: [N_CORES, 2, 120, 120] per-core PE blocks -> (argmin, min)."""
    # PSUM index = (c-within e, lane j), e-major.  Collapse banks and the
    # e-diagonal: G[j,k] = sum_banks sum_e out[(e,j),(e,k)].
    g6 = grams.reshape(N_CORES, 2, FG, NJ, FG, NJ).astype(np.float64)
    g24 = np.einsum("cnejek->cjk", g6)

    # Per full-batch index b: core c = b // BL, local l = b % BL.
    # Stream layout per core: x_(l,s) at 4*l+s, m1_l at 16+l, m2_l at 20+l.
    Gxx = np.empty((B, S, S), np.float64)   # sum_t x_s x_s'
    C1 = np.empty((B, S), np.float64)       # sum_t x_s m1
    C2 = np.empty((B, S), np.float64)
    M1 = np.empty((B,), np.float64)         # sum_t m1^2
    M2 = np.empty((B,), np.float64)
    for b in range(B):
        c, l = divmod(b, BL)
        gm = g24[c]
        xs = slice(S * l, S * l + S)
        Gxx[b] = gm[xs, xs]
        C1[b] = gm[xs, 16 + l]
        C2[b] = gm[xs, 20 + l]
        M1[b] = gm[16 + l, 16 + l]
        M2[b] = gm[20 + l, 20 + l]

    combos = np.array(list(itertools.product([0, 1], repeat=S)), np.float64)
    losses = np.zeros(len(combos), np.float64)
    with np.errstate(divide="ignore"):
        for w, cc, mm in ((combos, C1, M1), (1.0 - combos, C2, M2)):
            bq = np.einsum("ks,bst,kt->kb", w, Gxx, w)        # sum_t y^2
            aq = bq - 2.0 * (w @ cc.T) + mm[None, :]          # sum_t (y-m)^2
            losses += np.sum(10.0 * np.log10(aq + SNR_MAX * bq)
                             - 10.0 * np.log10(bq), axis=1)
    k = int(np.argmin(losses))
    return np.int32(k), np.float32(losses[k])


def _ensure_trace_hook_safe():
    """If BASS_TRACE is set but this image lacks antenv.axon_hooks, install a
    null hook module so run_bass_kernel_spmd degrades to an untraced run
    instead of crashing on the import."""
    try:
        import antenv.axon_hooks  # noqa: F401
    except ImportError:
        import types

        stub = types.ModuleType("antenv.axon_hooks")
        stub.get_axon_ntff_profile_hook = lambda: None
        stub.set_axon_ntff_profile_hook = lambda h: None
        sys.modules["antenv.axon_hooks"] = stub


def kernel(estimated_sources: np.ndarray, m1: np.ndarray, m2: np.ndarray):
    global LAST_RESULTS
    _ensure_trace_hook_safe()
    from concourse.bass_utils import run_bass_kernel_spmd

    x = np.ascontiguousarray(estimated_sources, dtype=np.float32)
    m1 = np.ascontiguousarray(m1, dtype=np.float32)
    m2 = np.ascontiguousarray(m2, dtype=np.float32)

    in_maps = []
    for c in range(N_CORES):
        sl = slice(BL * c, BL * (c + 1))
        in_maps.append({
            "x": np.ascontiguousarray(x[sl]),
            "m1": np.ascontiguousarray(m1[sl]),
            "m2": np.ascontiguousarray(m2[sl]),
        })

    nc = _get_nc()
    LAST_RESULTS = run_bass_kernel_spmd(nc, in_maps, list(range(N_CORES)))
    grams = np.stack([LAST_RESULTS.results[c]["g"] for c in range(N_CORES)])
    return _finish_host(grams)


# revision 16
# speedup vs baseline: 1.1334x; 1.1334x over previous
"""MixIT loss kernel for Trainium2 (8 NeuronCores, Bass/Tile) — v6.

Math: reference computes, for each of 16 assignment combinations k,
    mix[k,b,c,t] = sum_s A[k,c,s] * x[b,s,t]        (A tiny [16,2,4])
    loss[k] = sum_b [ snr(mix[k,b,0], m1[b]) + snr(mix[k,b,1], m2[b]) ]
and returns (argmin_k, min_k).  Everything reduces to the 6x6 Gram matrix
of the per-batch streams {x_0..x_3, m1, m2} over T=64000; the device
computes pairwise dot products, the host finishes the 16-way argmin.

v6 layout per core (4 batches = 24 streams; T = 128 partitions x 500
cols):
 - DMA (f32): m1/m2 as full-row transfers (2000B descriptors, ~94% HBM
   eff) on the scalar ring; x in tapered T-chunks on the sync ring so
   downstream stages can chase; the tiny last chunk shrinks the tail.
 - Re-layout: f32 tensor_copy za[p, j, c] -> zbt[p, c, j] (the fastest
   measured DVE pattern: strided 4B reads, contiguous 96B dst runs, 24
   lanes per op).  Work is split along c across DVE / GpSimd / ACT in
   per-chunk slices sized to each engine's measured rate; ACT gets one
   big op per chunk since each ACT op pays a ~1.4us pipe drain.
 - PE reads zbt as bf16 with NO cast: the operand is the high u16 half
   of each f32 word (= bf16 truncation, error < 0.4%, cancels in the
   final log-ratio; validated ~3e-8 end-to-end) via a uniform stride-2
   view — [128, 120] per group, so matmuls run 1-pass bf16 instead of
   2-pass fp32.  100 accumulating matmuls into 2 PSUM banks; bank A
   drains while the PE runs the small tail chunk.
 - Host sums the e-diagonal: G[j,k] = sum_banks sum_e out[(e,j),(e,k)].
"""

import itertools
import sys

import numpy as np

if "/opt/trn_rl_repo" not in sys.path:
    sys.path.insert(0, "/opt/trn_rl_repo")

N_CORES = 8
B = 32               # full batch
S = 4                # estimated sources
T = 64000
BL = B // N_CORES    # batches per core = 4
NJ = 6 * BL          # streams per core = 24 (16 x, 4 m1, 4 m2)
P = 128
COLS = T // P        # 500
FG = 5               # T-cols fused per matmul: FG*NJ = 120 <= 128
NG = COLS // FG      # 100 matmul groups
# T-chunks (cols, each % FG == 0, sum == COLS).  Tapered: big chunks
# amortize DMA descriptors, the small last chunk shrinks the PE tail.
X_CHUNKS = (140, 170, 105, 65, 20)
assert sum(X_CHUNKS) == COLS and all(c % FG == 0 for c in X_CHUNKS)
# Per-chunk copy split across engines, fractions of the chunk's columns;
# sized to measured rates DVE 2.0 / GpSimd 3.5 / ACT 3.6 ns/elem (ACT
# also issues 3 of the 4 chunk DMAs, so it gets a smaller share).
CP_DVE, CP_GPS = 0.50, 0.30           # ACT gets the rest
SNR_MAX = 30.0

_CACHE = {}
LAST_RESULTS = None  # BassKernelResults of the most recent run (for test harness)


def _build_nc():
    from concourse import bacc, bass, tile
    import concourse.mybir as mybir

    nc = bacc.Bacc("TRN2", target_bir_lowering=False, debug=False,
                   num_devices=N_CORES)
    f32 = mybir.dt.float32
    bf16 = mybir.dt.bfloat16
    x = nc.dram_tensor("x", [BL, S, T], f32, kind="ExternalInput")
    m1 = nc.dram_tensor("m1", [BL, T], f32, kind="ExternalInput")
    m2 = nc.dram_tensor("m2", [BL, T], f32, kind="ExternalInput")
    g = nc.dram_tensor("g", [2, FG * NJ, FG * NJ], f32, kind="ExternalOutput")

    grp_a_end = sum(X_CHUNKS[:-1]) // FG       # bank A: all but last chunk

    with tile.TileContext(nc) as tc:
        with (
            tc.tile_pool(name="za", bufs=1) as zapool,
            tc.tile_pool(name="zb", bufs=1) as zbpool,
            tc.tile_pool(name="ps", bufs=1, space=bass.MemorySpace.PSUM) as psp,
            tc.tile_pool(name="o", bufs=1) as opool,
        ):
            za = zapool.tile([P, NJ, COLS], f32, tag="za")
            zbt = zbpool.tile([P, COLS, NJ, 2], bf16, tag="zbt")
            zbf = zbt.bitcast(f32)             # [P, COLS, NJ, 1] f32 view
            acc_a = psp.tile([FG * NJ, FG * NJ], f32, tag="pa")
            acc_b = psp.tile([FG * NJ, FG * NJ], f32, tag="pb")

            xr = x.ap().rearrange("b s (p c) -> p (b s) c", p=P)
            m1r = m1.ap().rearrange("b (p c) -> p b c", p=P)
            m2r = m2.ap().rearrange("b (p c) -> p b c", p=P)

            def cp(eng, c0, c1):
                if c1 <= c0:
                    return
                dst = zbf[:, c0:c1, :, 0]
                src = za[:, :, c0:c1].transpose([0, 2, 1])
                if eng is nc.scalar:
                    eng.copy(dst, src)
                else:
                    eng.tensor_copy(dst, src)

            # All tensors T-chunked.  Sync carries x, scalar carries m1/m2
            # (so each chunk's 24 lanes finish near-together) plus its copy
            # slice.  Scalar's program interleaves next-chunk DMA issues
            # ahead of current-chunk copies so descriptor gen stays ahead.
            bounds = []
            c0 = 0
            for cq in X_CHUNKS:
                bounds.append((c0, cq))
                c0 += cq
            for q, (c0, cq) in enumerate(bounds):
                sl = slice(c0, c0 + cq)
                nc.sync.dma_start(out=za[:, 0:8, sl], in_=xr[:, 0:8, sl])
                nc.sync.dma_start(out=za[:, 8:16, sl], in_=xr[:, 8:16, sl])
                nc.scalar.dma_start(out=za[:, 16:20, sl], in_=m1r[:, :, sl])
                nc.scalar.dma_start(out=za[:, 20:24, sl], in_=m2r[:, :, sl])
                if q > 0:
                    p0, pq = bounds[q - 1]
                    dv = p0 + int(pq * CP_DVE)
                    gp = dv + int(pq * CP_GPS)
                    cp(nc.vector, p0, dv)
                    cp(nc.gpsimd, dv, gp)
                    cp(nc.scalar, gp, p0 + pq)
            p0, pq = bounds[-1]
            dv = p0 + int(pq * CP_DVE)
            gp = dv + int(pq * CP_GPS)
            cp(nc.vector, p0, dv)
            cp(nc.gpsimd, dv, gp)
            cp(nc.scalar, gp, p0 + pq)

            for grp in range(NG):
                # High u16 halves of 120 consecutive f32: [128, 120] bf16
                # at uniform element stride 2 — a 1-pass bf16 matmul.
                op = zbt[:, FG * grp:FG * (grp + 1), :, 1]
                acc = acc_a if grp < grp_a_end else acc_b
                nc.tensor.matmul(
                    acc[:, :], op, op,
                    start=(grp == 0 or grp == grp_a_end),
                    stop=(grp == grp_a_end - 1 or grp == NG - 1),
                )
                if grp == grp_a_end - 1:
                    # bank A done: drain it while the PE runs the tail chunk
                    gout_a = opool.tile([FG * NJ, FG * NJ], f32, tag="oa")
                    nc.vector.tensor_copy(gout_a[:, :], acc_a[:, :])
                    nc.sync.dma_start(out=g.ap()[0], in_=gout_a[:, :])
            gout_b = opool.tile([FG * NJ, FG * NJ], f32, tag="ob")
            nc.vector.tensor_copy(gout_b[:, :], acc_b[:, :])
            nc.sync.dma_start(out=g.ap()[1], in_=gout_b[:, :])
    nc.compile()
    return nc


def _get_nc():
    if "nc" not in _CACHE:
        _CACHE["nc"] = _build_nc()
    return _CACHE["nc"]


def _finish_host(grams: np.ndarray):
    """grams: [N_CORES, 2, 120, 120] per-core PE blocks -> (argmin, min)."""
    # PSUM index = (c-within e, lane j), e-major.  Collapse banks and the
    # e-diagonal: G[j,k] = sum_banks sum_e out[(e,j),(e,k)].
    g6 = grams.reshape(N_CORES, 2, FG, NJ, FG, NJ).astype(np.float64)
    g24 = np.einsum("cnejek->cjk", g6)

    # Per full-batch index b: core c = b // BL, local l = b % BL.
    # Stream layout per core: x_(l,s) at 4*l+s, m1_l at 16+l, m2_l at 20+l.
    Gxx = np.empty((B, S, S), np.float64)   # sum_t x_s x_s'
    C1 = np.empty((B, S), np.float64)       # sum_t x_s m1
    C2 = np.empty((B, S), np.float64)
    M1 = np.empty((B,), np.float64)         # sum_t m1^2
    M2 = np.empty((B,), np.float64)
    for b in range(B):
        c, l = divmod(b, BL)
        gm = g24[c]
        xs = slice(S * l, S * l + S)
        Gxx[b] = gm[xs, xs]
        C1[b] = gm[xs, 16 + l]
        C2[b] = gm[xs, 20 + l]
        M1[b] = gm[16 + l, 16 + l]
        M2[b] = gm[20 + l, 20 + l]

    combos = np.array(list(itertools.product([0, 1], repeat=S)), np.float64)
    losses = np.zeros(len(combos), np.float64)
    with np.errstate(divide="ignore"):
        for w, cc, mm in ((combos, C1, M1), (1.0 - combos, C2, M2)):
            bq = np.einsum("ks,bst,kt->kb", w, Gxx, w)        # sum_t y^2
            aq = bq - 2.0 * (w @ cc.T) + mm[None, :]          # sum_t (y-m)^2
            losses += np.sum(10.0 * np.log10(aq + SNR_MAX * bq)
                             - 10.0 * np.log10(bq), axis=1)
    k = int(np.argmin(losses))
    return np.int32(k), np.float32(losses[k])


def _ensure_trace_hook_safe():
    """If BASS_TRACE is set but this image lacks antenv.axon_hooks, install a
    null hook module so run_bass_kernel_spmd degrades to an untraced run
    instead of crashing on the import."""
    try:
        import antenv.axon_hooks  # noqa: F401
    except ImportError:
        import types

        stub = types.ModuleType("antenv.axon_hooks")
        stub.get_axon_ntff_profile_hook = lambda: None
        stub.set_axon_ntff_profile_hook = lambda h: None
        sys.modules["antenv.axon_hooks"] = stub


def kernel(estimated_sources: np.ndarray, m1: np.ndarray, m2: np.ndarray):
    global LAST_RESULTS
    _ensure_trace_hook_safe()
    from concourse.bass_utils import run_bass_kernel_spmd

    x = np.ascontiguousarray(estimated_sources, dtype=np.float32)
    m1 = np.ascontiguousarray(m1, dtype=np.float32)
    m2 = np.ascontiguousarray(m2, dtype=np.float32)

    in_maps = []
    for c in range(N_CORES):
        sl = slice(BL * c, BL * (c + 1))
        in_maps.append({
            "x": np.ascontiguousarray(x[sl]),
            "m1": np.ascontiguousarray(m1[sl]),
            "m2": np.ascontiguousarray(m2[sl]),
        })

    nc = _get_nc()
    LAST_RESULTS = run_bass_kernel_spmd(nc, in_maps, list(range(N_CORES)))
    grams = np.stack([LAST_RESULTS.results[c]["g"] for c in range(N_CORES)])
    return _finish_host(grams)


# revision 17
# speedup vs baseline: 1.1557x; 1.0197x over previous
"""MixIT loss kernel for Trainium2 (8 NeuronCores, Bass/Tile) — v6.

Math: reference computes, for each of 16 assignment combinations k,
    mix[k,b,c,t] = sum_s A[k,c,s] * x[b,s,t]        (A tiny [16,2,4])
    loss[k] = sum_b [ snr(mix[k,b,0], m1[b]) + snr(mix[k,b,1], m2[b]) ]
and returns (argmin_k, min_k).  Everything reduces to the 6x6 Gram matrix
of the per-batch streams {x_0..x_3, m1, m2} over T=64000; the device
computes pairwise dot products, the host finishes the 16-way argmin.

v6 layout per core (4 batches = 24 streams; T = 128 partitions x 500
cols):
 - DMA (f32): m1/m2 as full-row transfers (2000B descriptors, ~94% HBM
   eff) on the scalar ring; x in tapered T-chunks on the sync ring so
   downstream stages can chase; the tiny last chunk shrinks the tail.
 - Re-layout: f32 tensor_copy za[p, j, c] -> zbt[p, c, j] (the fastest
   measured DVE pattern: strided 4B reads, contiguous 96B dst runs, 24
   lanes per op).  Work is split along c across DVE / GpSimd / ACT in
   per-chunk slices sized to each engine's measured rate; ACT gets one
   big op per chunk since each ACT op pays a ~1.4us pipe drain.
 - PE reads zbt as bf16 with NO cast: the operand is the high u16 half
   of each f32 word (= bf16 truncation, error < 0.4%, cancels in the
   final log-ratio; validated ~3e-8 end-to-end) via a uniform stride-2
   view — [128, 120] per group, so matmuls run 1-pass bf16 instead of
   2-pass fp32.  100 accumulating matmuls into 2 PSUM banks; bank A
   drains while the PE runs the small tail chunk.
 - Host sums the e-diagonal: G[j,k] = sum_banks sum_e out[(e,j),(e,k)].
"""

import itertools
import sys

import numpy as np

if "/opt/trn_rl_repo" not in sys.path:
    sys.path.insert(0, "/opt/trn_rl_repo")

N_CORES = 8
B = 32               # full batch
S = 4                # estimated sources
T = 64000
BL = B // N_CORES    # batches per core = 4
NJ = 6 * BL          # streams per core = 24 (16 x, 4 m1, 4 m2)
P = 128
COLS = T // P        # 500
FG = 5               # T-cols fused per matmul: FG*NJ = 120 <= 128
NG = COLS // FG      # 100 matmul groups
# T-chunks (cols, each % FG == 0, sum == COLS).  Tapered: big chunks
# amortize DMA descriptors, the small last chunk shrinks the PE tail.
X_CHUNKS = (140, 170, 105, 65, 20)
assert sum(X_CHUNKS) == COLS and all(c % FG == 0 for c in X_CHUNKS)
# Per-chunk copy split across engines, fractions of the chunk's columns;
# sized to measured rates DVE 2.0 / GpSimd 3.5 / ACT 3.6 ns/elem (ACT
# also issues 3 of the 4 chunk DMAs, so it gets a smaller share).
CP_DVE, CP_GPS = 0.50, 0.30           # ACT gets the rest
SNR_MAX = 30.0

_CACHE = {}
LAST_RESULTS = None  # BassKernelResults of the most recent run (for test harness)


def _build_nc():
    from concourse import bacc, bass, tile
    import concourse.mybir as mybir

    nc = bacc.Bacc("TRN2", target_bir_lowering=False, debug=False,
                   num_devices=N_CORES)
    f32 = mybir.dt.float32
    bf16 = mybir.dt.bfloat16
    x = nc.dram_tensor("x", [BL, S, T], f32, kind="ExternalInput")
    m1 = nc.dram_tensor("m1", [BL, T], f32, kind="ExternalInput")
    m2 = nc.dram_tensor("m2", [BL, T], f32, kind="ExternalInput")
    g = nc.dram_tensor("g", [2, FG * NJ, FG * NJ], f32, kind="ExternalOutput")

    grp_a_end = sum(X_CHUNKS[:-1]) // FG       # bank A: all but last chunk

    with tile.TileContext(nc) as tc:
        with (
            tc.tile_pool(name="za", bufs=1) as zapool,
            tc.tile_pool(name="zb", bufs=1) as zbpool,
            tc.tile_pool(name="ps", bufs=1, space=bass.MemorySpace.PSUM) as psp,
            tc.tile_pool(name="o", bufs=1) as opool,
        ):
            za = zapool.tile([P, NJ, COLS], f32, tag="za")
            zbt = zbpool.tile([P, COLS, NJ, 2], bf16, tag="zbt")
            zbf = zbt.bitcast(f32)             # [P, COLS, NJ, 1] f32 view
            acc_a = psp.tile([FG * NJ, FG * NJ], f32, tag="pa")
            acc_b = psp.tile([FG * NJ, FG * NJ], f32, tag="pb")

            xr = x.ap().rearrange("b s (p c) -> p (b s) c", p=P)
            m1r = m1.ap().rearrange("b (p c) -> p b c", p=P)
            m2r = m2.ap().rearrange("b (p c) -> p b c", p=P)

            def cp(eng, c0, c1):
                if c1 <= c0:
                    return
                dst = zbf[:, c0:c1, :, 0]
                src = za[:, :, c0:c1].transpose([0, 2, 1])
                if eng is nc.scalar:
                    eng.copy(dst, src)
                else:
                    eng.tensor_copy(dst, src)

            # All tensors T-chunked.  Sync carries x, scalar carries m1/m2
            # (so each chunk's 24 lanes finish near-together) plus its copy
            # slice.  Scalar's program interleaves next-chunk DMA issues
            # ahead of current-chunk copies so descriptor gen stays ahead.
            bounds = []
            c0 = 0
            for cq in X_CHUNKS:
                bounds.append((c0, cq))
                c0 += cq
            for q, (c0, cq) in enumerate(bounds):
                sl = slice(c0, c0 + cq)
                nc.sync.dma_start(out=za[:, 0:12, sl], in_=xr[:, 0:12, sl])
                nc.scalar.dma_start(out=za[:, 12:16, sl],
                                    in_=xr[:, 12:16, sl])
                nc.scalar.dma_start(out=za[:, 16:20, sl], in_=m1r[:, :, sl])
                nc.scalar.dma_start(out=za[:, 20:24, sl], in_=m2r[:, :, sl])
                if q > 0:
                    p0, pq = bounds[q - 1]
                    dv = p0 + int(pq * CP_DVE)
                    gp = dv + int(pq * CP_GPS)
                    cp(nc.vector, p0, dv)
                    cp(nc.gpsimd, dv, gp)
                    cp(nc.scalar, gp, p0 + pq)
            p0, pq = bounds[-1]
            dv = p0 + int(pq * CP_DVE)
            gp = dv + int(pq * CP_GPS)
            cp(nc.vector, p0, dv)
            cp(nc.gpsimd, dv, gp)
            cp(nc.scalar, gp, p0 + pq)

            for grp in range(NG):
                # High u16 halves of 120 consecutive f32: [128, 120] bf16
                # at uniform element stride 2 — a 1-pass bf16 matmul.
                op = zbt[:, FG * grp:FG * (grp + 1), :, 1]
                acc = acc_a if grp < grp_a_end else acc_b
                nc.tensor.matmul(
                    acc[:, :], op, op,
                    start=(grp == 0 or grp == grp_a_end),
                    stop=(grp == grp_a_end - 1 or grp == NG - 1),
                )
                if grp == grp_a_end - 1:
                    # bank A done: drain it while the PE runs the tail chunk
                    gout_a = opool.tile([FG * NJ, FG * NJ], f32, tag="oa")
                    nc.vector.tensor_copy(gout_a[:, :], acc_a[:, :])
                    nc.sync.dma_start(out=g.ap()[0], in_=gout_a[:, :])
            gout_b = opool.tile([FG * NJ, FG * NJ], f32, tag="ob")
            nc.vector.tensor_copy(gout_b[:, :], acc_b[:, :])
            nc.sync.dma_start(out=g.ap()[1], in_=gout_b[:, :])
    nc.compile()
    return nc


def _get_nc():
    if "nc" not in _CACHE:
        _CACHE["nc"] = _build_nc()
    return _CACHE["nc"]


def _finish_host(grams: np.ndarray):
    """grams: [N_CORES, 2, 120, 120] per-core PE blocks -> (argmin, min)."""
    # PSUM index = (c-within e, lane j), e-major.  Collapse banks and the
    # e-diagonal: G[j,k] = sum_banks sum_e out[(e,j),(e,k)].
    g6 = grams.reshape(N_CORES, 2, FG, NJ, FG, NJ).astype(np.float64)
    g24 = np.einsum("cnejek->cjk", g6)

    # Per full-batch index b: core c = b // BL, local l = b % BL.
    # Stream layout per core: x_(l,s) at 4*l+s, m1_l at 16+l, m2_l at 20+l.
    Gxx = np.empty((B, S, S), np.float64)   # sum_t x_s x_s'
    C1 = np.empty((B, S), np.float64)       # sum_t x_s m1
    C2 = np.empty((B, S), np.float64)
    M1 = np.empty((B,), np.float64)         # sum_t m1^2
    M2 = np.empty((B,), np.float64)
    for b in range(B):
        c, l = divmod(b, BL)
        gm = g24[c]
        xs = slice(S * l, S * l + S)
        Gxx[b] = gm[xs, xs]
        C1[b] = gm[xs, 16 + l]
        C2[b] = gm[xs, 20 + l]
        M1[b] = gm[16 + l, 16 + l]
        M2[b] = gm[20 + l, 20 + l]

    combos = np.array(list(itertools.product([0, 1], repeat=S)), np.float64)
    losses = np.zeros(len(combos), np.float64)
    with np.errstate(divide="ignore"):
        for w, cc, mm in ((combos, C1, M1), (1.0 - combos, C2, M2)):
            bq = np.einsum("ks,bst,kt->kb", w, Gxx, w)        # sum_t y^2
            aq = bq - 2.0 * (w @ cc.T) + mm[None, :]          # sum_t (y-m)^2
            losses += np.sum(10.0 * np.log10(aq + SNR_MAX * bq)
                             - 10.0 * np.log10(bq), axis=1)
    k = int(np.argmin(losses))
    return np.int32(k), np.float32(losses[k])


def _ensure_trace_hook_safe():
    """If BASS_TRACE is set but this image lacks antenv.axon_hooks, install a
    null hook module so run_bass_kernel_spmd degrades to an untraced run
    instead of crashing on the import."""
    try:
        import antenv.axon_hooks  # noqa: F401
    except ImportError:
        import types

        stub = types.ModuleType("antenv.axon_hooks")
        stub.get_axon_ntff_profile_hook = lambda: None
        stub.set_axon_ntff_profile_hook = lambda h: None
        sys.modules["antenv.axon_hooks"] = stub


def kernel(estimated_sources: np.ndarray, m1: np.ndarray, m2: np.ndarray):
    global LAST_RESULTS
    _ensure_trace_hook_safe()
    from concourse.bass_utils import run_bass_kernel_spmd

    x = np.ascontiguousarray(estimated_sources, dtype=np.float32)
    m1 = np.ascontiguousarray(m1, dtype=np.float32)
    m2 = np.ascontiguousarray(m2, dtype=np.float32)

    in_maps = []
    for c in range(N_CORES):
        sl = slice(BL * c, BL * (c + 1))
        in_maps.append({
            "x": np.ascontiguousarray(x[sl]),
            "m1": np.ascontiguousarray(m1[sl]),
            "m2": np.ascontiguousarray(m2[sl]),
        })

    nc = _get_nc()
    LAST_RESULTS = run_bass_kernel_spmd(nc, in_maps, list(range(N_CORES)))
    grams = np.stack([LAST_RESULTS.results[c]["g"] for c in range(N_CORES)])
    return _finish_host(grams)


# revision 18
# speedup vs baseline: 1.1635x; 1.0068x over previous
"""MixIT loss kernel for Trainium2 (8 NeuronCores, Bass/Tile) — v6.

Math: reference computes, for each of 16 assignment combinations k,
    mix[k,b,c,t] = sum_s A[k,c,s] * x[b,s,t]        (A tiny [16,2,4])
    loss[k] = sum_b [ snr(mix[k,b,0], m1[b]) + snr(mix[k,b,1], m2[b]) ]
and returns (argmin_k, min_k).  Everything reduces to the 6x6 Gram matrix
of the per-batch streams {x_0..x_3, m1, m2} over T=64000; the device
computes pairwise dot products, the host finishes the 16-way argmin.

v6 layout per core (4 batches = 24 streams; T = 128 partitions x 500
cols):
 - DMA (f32): m1/m2 as full-row transfers (2000B descriptors, ~94% HBM
   eff) on the scalar ring; x in tapered T-chunks on the sync ring so
   downstream stages can chase; the tiny last chunk shrinks the tail.
 - Re-layout: f32 tensor_copy za[p, j, c] -> zbt[p, c, j] (the fastest
   measured DVE pattern: strided 4B reads, contiguous 96B dst runs, 24
   lanes per op).  Work is split along c across DVE / GpSimd / ACT in
   per-chunk slices sized to each engine's measured rate; ACT gets one
   big op per chunk since each ACT op pays a ~1.4us pipe drain.
 - PE reads zbt as bf16 with NO cast: the operand is the high u16 half
   of each f32 word (= bf16 truncation, error < 0.4%, cancels in the
   final log-ratio; validated ~3e-8 end-to-end) via a uniform stride-2
   view — [128, 120] per group, so matmuls run 1-pass bf16 instead of
   2-pass fp32.  100 accumulating matmuls into 2 PSUM banks; bank A
   drains while the PE runs the small tail chunk.
 - Host sums the e-diagonal: G[j,k] = sum_banks sum_e out[(e,j),(e,k)].
"""

import itertools
import sys

import numpy as np

if "/opt/trn_rl_repo" not in sys.path:
    sys.path.insert(0, "/opt/trn_rl_repo")

N_CORES = 8
B = 32               # full batch
S = 4                # estimated sources
T = 64000
BL = B // N_CORES    # batches per core = 4
NJ = 6 * BL          # streams per core = 24 (16 x, 4 m1, 4 m2)
P = 128
COLS = T // P        # 500
FG = 5               # T-cols fused per matmul: FG*NJ = 120 <= 128
NG = COLS // FG      # 100 matmul groups
# T-chunks (cols, each % FG == 0, sum == COLS).  Tapered: big chunks
# amortize DMA descriptors, the small last chunk shrinks the PE tail.
X_CHUNKS = (100, 220, 160, 20)
assert sum(X_CHUNKS) == COLS and all(c % FG == 0 for c in X_CHUNKS)
# Per-chunk copy split across engines, fractions of the chunk's columns;
# sized to measured rates DVE 2.0 / GpSimd 3.5 / ACT 3.6 ns/elem (ACT
# also issues 3 of the 4 chunk DMAs, so it gets a smaller share).
CP_DVE, CP_GPS = 0.50, 0.20           # ACT gets the rest
SNR_MAX = 30.0

_CACHE = {}
LAST_RESULTS = None  # BassKernelResults of the most recent run (for test harness)


def _build_nc():
    from concourse import bacc, bass, tile
    import concourse.mybir as mybir

    nc = bacc.Bacc("TRN2", target_bir_lowering=False, debug=False,
                   num_devices=N_CORES)
    f32 = mybir.dt.float32
    bf16 = mybir.dt.bfloat16
    x = nc.dram_tensor("x", [BL, S, T], f32, kind="ExternalInput")
    m1 = nc.dram_tensor("m1", [BL, T], f32, kind="ExternalInput")
    m2 = nc.dram_tensor("m2", [BL, T], f32, kind="ExternalInput")
    g = nc.dram_tensor("g", [2, FG * NJ, FG * NJ], f32, kind="ExternalOutput")

    grp_a_end = sum(X_CHUNKS[:2]) // FG        # bank A: first two chunks

    with tile.TileContext(nc) as tc:
        with (
            tc.tile_pool(name="za", bufs=1) as zapool,
            tc.tile_pool(name="zb", bufs=1) as zbpool,
            tc.tile_pool(name="ps", bufs=1, space=bass.MemorySpace.PSUM) as psp,
            tc.tile_pool(name="o", bufs=1) as opool,
        ):
            za = zapool.tile([P, NJ, COLS], f32, tag="za")
            zbt = zbpool.tile([P, COLS, NJ, 2], bf16, tag="zbt")
            zbf = zbt.bitcast(f32)             # [P, COLS, NJ, 1] f32 view
            acc_a = psp.tile([FG * NJ, FG * NJ], f32, tag="pa")
            acc_b = psp.tile([FG * NJ, FG * NJ], f32, tag="pb")

            xr = x.ap().rearrange("b s (p c) -> p (b s) c", p=P)
            m1r = m1.ap().rearrange("b (p c) -> p b c", p=P)
            m2r = m2.ap().rearrange("b (p c) -> p b c", p=P)

            def cp(eng, c0, c1):
                if c1 <= c0:
                    return
                dst = zbf[:, c0:c1, :, 0]
                src = za[:, :, c0:c1].transpose([0, 2, 1])
                if eng is nc.scalar:
                    eng.copy(dst, src)
                else:
                    eng.tensor_copy(dst, src)

            def chunk_copies(b):
                # Split big chunks into two c-halves per engine so the PE
                # can start on the first half while the second copies.
                p0, pq = b
                halves = [(p0, pq)] if pq < 150 else \
                    [(p0, pq - pq // 2), (p0 + pq - pq // 2, pq // 2)]
                for h0, hq in halves:
                    dv = h0 + int(hq * CP_DVE)
                    gp = dv + int(hq * CP_GPS)
                    cp(nc.vector, h0, dv)
                    cp(nc.gpsimd, dv, gp)
                    cp(nc.scalar, gp, h0 + hq)

            # All tensors T-chunked.  Sync carries x lanes 0:12, scalar
            # carries x 12:16 + m1 + m2 (equal bytes and descriptor sizes
            # per ring, so a chunk's 24 lanes finish near-together) plus
            # its copy slice.  Scalar's program interleaves next-chunk DMA
            # issues ahead of current-chunk copies.
            bounds = []
            c0 = 0
            for cq in X_CHUNKS:
                bounds.append((c0, cq))
                c0 += cq
            for q, (c0, cq) in enumerate(bounds):
                sl = slice(c0, c0 + cq)
                nc.sync.dma_start(out=za[:, 0:12, sl], in_=xr[:, 0:12, sl])
                nc.scalar.dma_start(out=za[:, 12:16, sl],
                                    in_=xr[:, 12:16, sl])
                nc.scalar.dma_start(out=za[:, 16:20, sl], in_=m1r[:, :, sl])
                nc.scalar.dma_start(out=za[:, 20:24, sl], in_=m2r[:, :, sl])
                if q > 0:
                    chunk_copies(bounds[q - 1])
            chunk_copies(bounds[-1])

            for grp in range(NG):
                # High u16 halves of 120 consecutive f32: [128, 120] bf16
                # at uniform element stride 2 — a 1-pass bf16 matmul.
                op = zbt[:, FG * grp:FG * (grp + 1), :, 1]
                acc = acc_a if grp < grp_a_end else acc_b
                nc.tensor.matmul(
                    acc[:, :], op, op,
                    start=(grp == 0 or grp == grp_a_end),
                    stop=(grp == grp_a_end - 1 or grp == NG - 1),
                )
                if grp == grp_a_end - 1:
                    # bank A done: drain it while the PE runs the tail chunk
                    gout_a = opool.tile([FG * NJ, FG * NJ], f32, tag="oa")
                    nc.vector.tensor_copy(gout_a[:, :], acc_a[:, :])
                    nc.sync.dma_start(out=g.ap()[0], in_=gout_a[:, :])
            gout_b = opool.tile([FG * NJ, FG * NJ], f32, tag="ob")
            nc.vector.tensor_copy(gout_b[:, :], acc_b[:, :])
            nc.sync.dma_start(out=g.ap()[1], in_=gout_b[:, :])
    nc.compile()
    return nc


def _get_nc():
    if "nc" not in _CACHE:
        _CACHE["nc"] = _build_nc()
    return _CACHE["nc"]


def _finish_host(grams: np.ndarray):
    """grams: [N_CORES, 2, 120, 120] per-core PE blocks -> (argmin, min)."""
    # PSUM index = (c-within e, lane j), e-major.  Collapse banks and the
    # e-diagonal: G[j,k] = sum_banks sum_e out[(e,j),(e,k)].
    g6 = grams.reshape(N_CORES, 2, FG, NJ, FG, NJ).astype(np.float64)
    g24 = np.einsum("cnejek->cjk", g6)

    # Per full-batch index b: core c = b // BL, local l = b % BL.
    # Stream layout per core: x_(l,s) at 4*l+s, m1_l at 16+l, m2_l at 20+l.
    Gxx = np.empty((B, S, S), np.float64)   # sum_t x_s x_s'
    C1 = np.empty((B, S), np.float64)       # sum_t x_s m1
    C2 = np.empty((B, S), np.float64)
    M1 = np.empty((B,), np.float64)         # sum_t m1^2
    M2 = np.empty((B,), np.float64)
    for b in range(B):
        c, l = divmod(b, BL)
        gm = g24[c]
        xs = slice(S * l, S * l + S)
        Gxx[b] = gm[xs, xs]
        C1[b] = gm[xs, 16 + l]
        C2[b] = gm[xs, 20 + l]
        M1[b] = gm[16 + l, 16 + l]
        M2[b] = gm[20 + l, 20 + l]

    combos = np.array(list(itertools.product([0, 1], repeat=S)), np.float64)
    losses = np.zeros(len(combos), np.float64)
    with np.errstate(divide="ignore"):
        for w, cc, mm in ((combos, C1, M1), (1.0 - combos, C2, M2)):
            bq = np.einsum("ks,bst,kt->kb", w, Gxx, w)        # sum_t y^2
            aq = bq - 2.0 * (w @ cc.T) + mm[None, :]          # sum_t (y-m)^2
            losses += np.sum(10.0 * np.log10(aq + SNR_MAX * bq)
                             - 10.0 * np.log10(bq), axis=1)
    k = int(np.argmin(losses))
    return np.int32(k), np.float32(losses[k])


def _ensure_trace_hook_safe():
    """If BASS_TRACE is set but this image lacks antenv.axon_hooks, install a
    null hook module so run_bass_kernel_spmd degrades to an untraced run
    instead of crashing on the import."""
    try:
        import antenv.axon_hooks  # noqa: F401
    except ImportError:
        import types

        stub = types.ModuleType("antenv.axon_hooks")
        stub.get_axon_ntff_profile_hook = lambda: None
        stub.set_axon_ntff_profile_hook = lambda h: None
        sys.modules["antenv.axon_hooks"] = stub


def kernel(estimated_sources: np.ndarray, m1: np.ndarray, m2: np.ndarray):
    global LAST_RESULTS
    _ensure_trace_hook_safe()
    from concourse.bass_utils import run_bass_kernel_spmd

    x = np.ascontiguousarray(estimated_sources, dtype=np.float32)
    m1 = np.ascontiguousarray(m1, dtype=np.float32)
    m2 = np.ascontiguousarray(m2, dtype=np.float32)

    in_maps = []
    for c in range(N_CORES):
        sl = slice(BL * c, BL * (c + 1))
        in_maps.append({
            "x": np.ascontiguousarray(x[sl]),
            "m1": np.ascontiguousarray(m1[sl]),
            "m2": np.ascontiguousarray(m2[sl]),
        })

    nc = _get_nc()
    LAST_RESULTS = run_bass_kernel_spmd(nc, in_maps, list(range(N_CORES)))
    grams = np.stack([LAST_RESULTS.results[c]["g"] for c in range(N_CORES)])
    return _finish_host(grams)


# revision 19
# speedup vs baseline: 1.1651x; 1.0014x over previous
"""MixIT loss kernel for Trainium2 (8 NeuronCores, Bass/Tile) — v6.

Math: reference computes, for each of 16 assignment combinations k,
    mix[k,b,c,t] = sum_s A[k,c,s] * x[b,s,t]        (A tiny [16,2,4])
    loss[k] = sum_b [ snr(mix[k,b,0], m1[b]) + snr(mix[k,b,1], m2[b]) ]
and returns (argmin_k, min_k).  Everything reduces to the 6x6 Gram matrix
of the per-batch streams {x_0..x_3, m1, m2} over T=64000; the device
computes pairwise dot products, the host finishes the 16-way argmin.

v6 layout per core (4 batches = 24 streams; T = 128 partitions x 500
cols):
 - DMA (f32): m1/m2 as full-row transfers (2000B descriptors, ~94% HBM
   eff) on the scalar ring; x in tapered T-chunks on the sync ring so
   downstream stages can chase; the tiny last chunk shrinks the tail.
 - Re-layout: f32 tensor_copy za[p, j, c] -> zbt[p, c, j] (the fastest
   measured DVE pattern: strided 4B reads, contiguous 96B dst runs, 24
   lanes per op).  Work is split along c across DVE / GpSimd / ACT in
   per-chunk slices sized to each engine's measured rate; ACT gets one
   big op per chunk since each ACT op pays a ~1.4us pipe drain.
 - PE reads zbt as bf16 with NO cast: the operand is the high u16 half
   of each f32 word (= bf16 truncation, error < 0.4%, cancels in the
   final log-ratio; validated ~3e-8 end-to-end) via a uniform stride-2
   view — [128, 120] per group, so matmuls run 1-pass bf16 instead of
   2-pass fp32.  100 accumulating matmuls into 2 PSUM banks; bank A
   drains while the PE runs the small tail chunk.
 - Host sums the e-diagonal: G[j,k] = sum_banks sum_e out[(e,j),(e,k)].
"""

import itertools
import sys

import numpy as np

if "/opt/trn_rl_repo" not in sys.path:
    sys.path.insert(0, "/opt/trn_rl_repo")

N_CORES = 8
B = 32               # full batch
S = 4                # estimated sources
T = 64000
BL = B // N_CORES    # batches per core = 4
NJ = 6 * BL          # streams per core = 24 (16 x, 4 m1, 4 m2)
P = 128
COLS = T // P        # 500
FG = 5               # T-cols fused per matmul: FG*NJ = 120 <= 128
NG = COLS // FG      # 100 matmul groups
# T-chunks (cols, each % FG == 0, sum == COLS).  Tapered: big chunks
# amortize DMA descriptors, the small last chunk shrinks the PE tail.
X_CHUNKS = (100, 220, 160, 20)
assert sum(X_CHUNKS) == COLS and all(c % FG == 0 for c in X_CHUNKS)
# Per-chunk copy split across engines, fractions of the chunk's columns;
# sized to measured rates DVE 2.0 / GpSimd 3.5 / ACT 3.6 ns/elem (ACT
# also issues 3 of the 4 chunk DMAs, so it gets a smaller share).
CP_DVE, CP_GPS = 0.50, 0.20           # ACT gets the rest
SNR_MAX = 30.0

_CACHE = {}
LAST_RESULTS = None  # BassKernelResults of the most recent run (for test harness)


def _build_nc():
    from concourse import bacc, bass, tile
    import concourse.mybir as mybir

    nc = bacc.Bacc("TRN2", target_bir_lowering=False, debug=False,
                   num_devices=N_CORES)
    f32 = mybir.dt.float32
    bf16 = mybir.dt.bfloat16
    z = nc.dram_tensor("z", [NJ, T], f32, kind="ExternalInput")
    g = nc.dram_tensor("g", [2, FG * NJ, FG * NJ], f32, kind="ExternalOutput")

    grp_a_end = sum(X_CHUNKS[:2]) // FG        # bank A: first two chunks

    with tile.TileContext(nc) as tc:
        with (
            tc.tile_pool(name="za", bufs=1) as zapool,
            tc.tile_pool(name="zb", bufs=1) as zbpool,
            tc.tile_pool(name="ps", bufs=1, space=bass.MemorySpace.PSUM) as psp,
            tc.tile_pool(name="o", bufs=1) as opool,
        ):
            za = zapool.tile([P, NJ, COLS], f32, tag="za")
            zbt = zbpool.tile([P, COLS, NJ, 2], bf16, tag="zbt")
            zbf = zbt.bitcast(f32)             # [P, COLS, NJ, 1] f32 view
            acc_a = psp.tile([FG * NJ, FG * NJ], f32, tag="pa")
            acc_b = psp.tile([FG * NJ, FG * NJ], f32, tag="pb")

            zr = z.ap().rearrange("j (p c) -> p j c", p=P)

            def cp(eng, c0, c1):
                if c1 <= c0:
                    return
                dst = zbf[:, c0:c1, :, 0]
                src = za[:, :, c0:c1].transpose([0, 2, 1])
                if eng is nc.scalar:
                    eng.copy(dst, src)
                else:
                    eng.tensor_copy(dst, src)

            def chunk_copies(b):
                # Split big chunks into two c-halves per engine so the PE
                # can start on the first half while the second copies.
                p0, pq = b
                halves = [(p0, pq)] if pq < 150 else \
                    [(p0, pq - pq // 2), (p0 + pq - pq // 2, pq // 2)]
                for h0, hq in halves:
                    dv = h0 + int(hq * CP_DVE)
                    gp = dv + int(hq * CP_GPS)
                    cp(nc.vector, h0, dv)
                    cp(nc.gpsimd, dv, gp)
                    cp(nc.scalar, gp, h0 + hq)

            # One stacked tensor, T-chunked: ONE DMA per ring per chunk
            # (12 lanes each, equal bytes and descriptor sizes) so a
            # chunk's lanes complete together with no per-ring FIFO
            # serialization.  Scalar's program interleaves next-chunk DMA
            # issues ahead of current-chunk copies.
            bounds = []
            c0 = 0
            for cq in X_CHUNKS:
                bounds.append((c0, cq))
                c0 += cq
            for q, (c0, cq) in enumerate(bounds):
                sl = slice(c0, c0 + cq)
                nc.sync.dma_start(out=za[:, 0:12, sl], in_=zr[:, 0:12, sl])
                nc.scalar.dma_start(out=za[:, 12:24, sl],
                                    in_=zr[:, 12:24, sl])
                if q > 0:
                    chunk_copies(bounds[q - 1])
            chunk_copies(bounds[-1])

            for grp in range(NG):
                # High u16 halves of 120 consecutive f32: [128, 120] bf16
                # at uniform element stride 2 — a 1-pass bf16 matmul.
                op = zbt[:, FG * grp:FG * (grp + 1), :, 1]
                acc = acc_a if grp < grp_a_end else acc_b
                nc.tensor.matmul(
                    acc[:, :], op, op,
                    start=(grp == 0 or grp == grp_a_end),
                    stop=(grp == grp_a_end - 1 or grp == NG - 1),
                )
                if grp == grp_a_end - 1:
                    # bank A done: drain it while the PE runs the tail chunk
                    gout_a = opool.tile([FG * NJ, FG * NJ], f32, tag="oa")
                    nc.vector.tensor_copy(gout_a[:, :], acc_a[:, :])
                    nc.sync.dma_start(out=g.ap()[0], in_=gout_a[:, :])
            gout_b = opool.tile([FG * NJ, FG * NJ], f32, tag="ob")
            nc.vector.tensor_copy(gout_b[:, :], acc_b[:, :])
            nc.sync.dma_start(out=g.ap()[1], in_=gout_b[:, :])
    nc.compile()
    return nc


def _get_nc():
    if "nc" not in _CACHE:
        _CACHE["nc"] = _build_nc()
    return _CACHE["nc"]


def _finish_host(grams: np.ndarray):
    """grams: [N_CORES, 2, 120, 120] per-core PE blocks -> (argmin, min)."""
    # PSUM index = (c-within e, lane j), e-major.  Collapse banks and the
    # e-diagonal: G[j,k] = sum_banks sum_e out[(e,j),(e,k)].
    g6 = grams.reshape(N_CORES, 2, FG, NJ, FG, NJ).astype(np.float64)
    g24 = np.einsum("cnejek->cjk", g6)

    # Per full-batch index b: core c = b // BL, local l = b % BL.
    # Stream layout per core: x_(l,s) at 4*l+s, m1_l at 16+l, m2_l at 20+l.
    Gxx = np.empty((B, S, S), np.float64)   # sum_t x_s x_s'
    C1 = np.empty((B, S), np.float64)       # sum_t x_s m1
    C2 = np.empty((B, S), np.float64)
    M1 = np.empty((B,), np.float64)         # sum_t m1^2
    M2 = np.empty((B,), np.float64)
    for b in range(B):
        c, l = divmod(b, BL)
        gm = g24[c]
        xs = slice(S * l, S * l + S)
        Gxx[b] = gm[xs, xs]
        C1[b] = gm[xs, 16 + l]
        C2[b] = gm[xs, 20 + l]
        M1[b] = gm[16 + l, 16 + l]
        M2[b] = gm[20 + l, 20 + l]

    combos = np.array(list(itertools.product([0, 1], repeat=S)), np.float64)
    losses = np.zeros(len(combos), np.float64)
    with np.errstate(divide="ignore"):
        for w, cc, mm in ((combos, C1, M1), (1.0 - combos, C2, M2)):
            bq = np.einsum("ks,bst,kt->kb", w, Gxx, w)        # sum_t y^2
            aq = bq - 2.0 * (w @ cc.T) + mm[None, :]          # sum_t (y-m)^2
            losses += np.sum(10.0 * np.log10(aq + SNR_MAX * bq)
                             - 10.0 * np.log10(bq), axis=1)
    k = int(np.argmin(losses))
    return np.int32(k), np.float32(losses[k])


def _ensure_trace_hook_safe():
    """If BASS_TRACE is set but this image lacks antenv.axon_hooks, install a
    null hook module so run_bass_kernel_spmd degrades to an untraced run
    instead of crashing on the import."""
    try:
        import antenv.axon_hooks  # noqa: F401
    except ImportError:
        import types

        stub = types.ModuleType("antenv.axon_hooks")
        stub.get_axon_ntff_profile_hook = lambda: None
        stub.set_axon_ntff_profile_hook = lambda h: None
        sys.modules["antenv.axon_hooks"] = stub


def kernel(estimated_sources: np.ndarray, m1: np.ndarray, m2: np.ndarray):
    global LAST_RESULTS
    _ensure_trace_hook_safe()
    from concourse.bass_utils import run_bass_kernel_spmd

    x = np.asarray(estimated_sources, dtype=np.float32)
    m1 = np.asarray(m1, dtype=np.float32)
    m2 = np.asarray(m2, dtype=np.float32)

    in_maps = []
    for c in range(N_CORES):
        sl = slice(BL * c, BL * (c + 1))
        z = np.concatenate([x[sl].reshape(4 * S, T), m1[sl], m2[sl]], axis=0)
        in_maps.append({"z": np.ascontiguousarray(z)})

    nc = _get_nc()
    LAST_RESULTS = run_bass_kernel_spmd(nc, in_maps, list(range(N_CORES)))
    grams = np.stack([LAST_RESULTS.results[c]["g"] for c in range(N_CORES)])
    return _finish_host(grams)
